# revision 1
# baseline (speedup 1.0000x reference)
"""Trainium2 Bass kernel for nn_DeepBKT (4-layer DeepBKT-style transformer).

Sharding: pure data-parallel over batch. B=32 sequences -> 8 NeuronCores x 4
sequences. Weights replicated. No collectives.

Per-core kernel design notes:
  - Activations kept in two orientations: canonical state x is seq-major
    [i(part), d(free)] (LayerNorm reduces over free dim); matmul contractions
    need the contracted dim on partitions, so x is PE-transposed to xT
    [d, i] once per phase.
  - qk projection emits feature-major qkT [d, i]; v projection emits
    seq-major v [j, d] -- both orientations fall out of choosing which
    operand is stationary, no extra transposes.
  - Attention scores are computed transposed, eT[j, i] (one matmul per
    (head, j-tile), N = i-range >= j-tile start: causal block skipping).
    Softmax runs without max-subtraction (scores bounded, |s| < ~20).
  - Softmax denominators come for free from the PV matmul by appending a
    ones-column to v (stationary [128, 65] per head); row 64 of the PV psum
    is sum_j e[j, i].
  - All big matmuls run in float32r (1 cyc/row at N>=256, ~1.5e-4 matmul
    rel err vs 4 cyc/row for fp32).
"""

import sys

for _p in ("/opt/trn_rl_repo",):
    if _p not in sys.path:
        sys.path.insert(0, _p)

import numpy as np

import concourse.bacc as bacc
import concourse.bass as bass
import concourse.tile as tile
import concourse.mybir as mybir
from concourse.masks import make_identity

# Raise the Tile SBUF cap: 192KB/partition default leaves 16KB/part unused on
# trn2 (224KB phys / 208KB usable).
import concourse.tile_utils as tile_utils

tile_utils.max_sbuf_usage = 208 * 1024

F32 = mybir.dt.float32
F32R = mybir.dt.float32r
BF16 = mybir.dt.bfloat16
FFN_BF16 = False
FFN_DT = BF16 if FFN_BF16 else F32R
AF = mybir.ActivationFunctionType
ALU = mybir.AluOpType

P = 128
S, D, H, FF = 512, 512, 8, 2048
DK = D // H  # 64
NT = S // P  # 4 i/j tiles
DT = D // P  # 4 d tiles
NKF = FF // P  # 16 ff tiles
EPS = 1e-5
NEG_BIG = -1e30
N_CORES = 8


def build(L=4, NB=4, fast=True):
    """Build the per-core Bass kernel. fast=True assumes zero biases and
    unit LN gains (checked by the host against the actual input values)."""
    nc = bacc.Bacc("TRN2", target_bir_lowering=False, debug=False,
                   num_devices=N_CORES)

    q_d = nc.dram_tensor("q", [NB, S, D], F32, kind="ExternalInput")
    qa_d = nc.dram_tensor("qa", [NB, S, D], F32, kind="ExternalInput")
    pid_d = nc.dram_tensor("pid", [NB, S, S], F32, kind="ExternalInput")
    fr_d = nc.dram_tensor("fr", [NB, S], F32, kind="ExternalInput")
    pos_d = nc.dram_tensor("pos", [S, D], F32, kind="ExternalInput")
    wk_d = nc.dram_tensor("Wk", [L, D, D], F32R, kind="ExternalInput")
    wv_d = nc.dram_tensor("Wv", [L, D, D], F32R, kind="ExternalInput")
    wo_d = nc.dram_tensor("Wo", [L, D, D], F32R, kind="ExternalInput")
    w1_d = nc.dram_tensor("W1", [L, D, FF], FFN_DT, kind="ExternalInput")
    w2_d = nc.dram_tensor("W2", [L, FF, D], FFN_DT, kind="ExternalInput")
    if not fast:
        bk_d = nc.dram_tensor("bk", [L, D], F32, kind="ExternalInput")
        bv_d = nc.dram_tensor("bv", [L, D], F32, kind="ExternalInput")
        bo_d = nc.dram_tensor("bo", [L, D], F32, kind="ExternalInput")
        b1_d = nc.dram_tensor("b1", [L, FF], F32, kind="ExternalInput")
        b2_d = nc.dram_tensor("b2", [L, D], F32, kind="ExternalInput")
        g1_d = nc.dram_tensor("g1", [L, D], F32, kind="ExternalInput")
        gb1_d = nc.dram_tensor("gb1", [L, D], F32, kind="ExternalInput")
        g2_d = nc.dram_tensor("g2", [L, D], F32, kind="ExternalInput")
        gb2_d = nc.dram_tensor("gb2", [L, D], F32, kind="ExternalInput")
    out_d = nc.dram_tensor("out", [NB, S, D], F32, kind="ExternalOutput")

    with tile.TileContext(nc) as tc:
        with (
            tc.tile_pool(name="const", bufs=1) as constp,
            tc.tile_pool(name="state", bufs=18 if fast else 12) as statep,
            tc.tile_pool(name="big", bufs=6 if fast else 4) as bigp,
            tc.tile_pool(name="med", bufs=22 if fast else 12) as medp,
            tc.tile_pool(name="w3", bufs=4) as w3p,
            tc.tile_pool(name="w1", bufs=2) as w1p,
            tc.tile_pool(name="w2", bufs=2) as w2p,
            tc.tile_pool(name="small", bufs=8) as smallp,
            tc.tile_pool(name="frsp", bufs=NB) as frsp,
            tc.tile_pool(name="ps", bufs=8, space="PSUM") as psp,
            tc.tile_pool(name="dram", bufs=1, space="DRAM") as dramp,
        ):
            ident = constp.tile([P, P], F32, tag="ident")
            make_identity(nc, ident)
            eps_t = constp.tile([P, 1], F32, tag="eps")
            nc.vector.memset(eps_t, EPS)
            eps37 = constp.tile([P, 1], F32, tag="eps37")
            nc.vector.memset(eps37, 1e-37)
            ones32 = constp.tile([P, NT * H], F32, tag="ones32")
            nc.vector.memset(ones32, 1.0)

            teT_dram = dramp.tile([NB, S, S], F32, tag="teT_d")
            yT_dram = dramp.tile([NB, S, D], F32R, tag="yT_d")

            def big_tile(dt_, cols=D):
                return bigp.tile([P, NT, cols], dt_, tag="big", name="bigt")

            def med_tile(dt_, cols=D):
                return medp.tile([P, cols], dt_, tag="med", name="medt")

            def transpose_512(src_of_it, out_dt):
                """src_of_it(it) -> AP [128, 512] seq-major tiles.
                Returns one [128, NT, 512] tile = transposed (feature-major)."""
                dst = big_tile(out_dt)
                for c in range(DT):
                    ps = psp.tile([P, S], F32, tag="psC")
                    for it in range(NT):
                        nc.tensor.transpose(
                            ps[:, it * P:(it + 1) * P],
                            src_of_it(it)[:, c * P:(c + 1) * P],
                            ident,
                        )
                    nc.scalar.copy(out=dst[:, c, :], in_=ps[:])
                return dst

            def ln_apply(t, rowsum, dst, g_bc=None, b_bc=None):
                """LayerNorm over free dim: t [128,512] f32 (pre-LN values),
                rowsum [128,1] = sum over free. Writes normalized into dst."""
                mean_neg = smallp.tile([P, 1], F32, tag="mneg")
                nc.scalar.mul(out=mean_neg, in_=rowsum, mul=-1.0 / D)
                var_s = smallp.tile([P, 1], F32, tag="vars")
                # dst used as throwaway scratch for the squares
                nc.scalar.activation(out=dst, in_=t, func=AF.Square,
                                     bias=mean_neg, scale=1.0,
                                     accum_out=var_s)
                std = smallp.tile([P, 1], F32, tag="std")
                nc.scalar.activation(out=std, in_=var_s, func=AF.Sqrt,
                                     bias=eps_t, scale=1.0 / D)
                rstd = smallp.tile([P, 1], F32, tag="rstd")
                nc.vector.reciprocal(out=rstd, in_=std)
                nc.vector.tensor_scalar(out=dst, in0=t, scalar1=mean_neg,
                                        scalar2=rstd, op0=ALU.add,
                                        op1=ALU.mult)
                if g_bc is not None:
                    nc.vector.tensor_mul(out=dst, in0=dst, in1=g_bc)
                if b_bc is not None:
                    nc.vector.tensor_add(out=dst, in0=dst, in1=b_bc)

            def bcast_row(src_row, cols=D):
                """Broadcast [1, cols] sbuf row to [128, cols] via PE."""
                onecol = constp.tile([1, P], F32, tag="onecol")
                nc.vector.memset(onecol, 1.0)
                ps = psp.tile([P, cols], F32, tag="psC")
                nc.tensor.matmul(ps[:], onecol[:], src_row, start=True,
                                 stop=True)
                dst = medp.tile([P, cols], F32, tag="bc", bufs=10, name="bct")
                nc.vector.tensor_copy(out=dst[:], in_=ps[:])
                return dst

            # ============ per-sequence init ============
            x_tiles = {}   # b -> list of NT state APs [128, 512] f32 (seq-major)
            frs = {}
            pos_t = big_tile(F32)
            nc.sync.dma_start(
                out=pos_t[:],
                in_=pos_d[:].rearrange("(it p) d -> p it d", p=P))

            for b in range(NB):
                # x = q + pos
                qt = big_tile(F32)
                nc.sync.dma_start(
                    out=qt[:], in_=q_d[b].rearrange("(it p) d -> p it d", p=P))
                xb = []
                for it in range(NT):
                    xt = statep.tile([P, D], F32, tag="x")
                    nc.vector.tensor_add(out=xt[:], in0=qt[:, it, :],
                                         in1=pos_t[:, it, :])
                    xb.append(xt)
                x_tiles[b] = xb

                # y = qa + pos; yT -> DRAM scratch (f32r)
                yt = big_tile(F32)
                nc.sync.dma_start(
                    out=yt[:], in_=qa_d[b].rearrange("(it p) d -> p it d", p=P))
                for it in range(NT):
                    nc.vector.tensor_add(out=yt[:, it, :], in0=yt[:, it, :],
                                         in1=pos_t[:, it, :])
                yT = transpose_512(lambda it: yt[:, it, :], F32R)
                nc.sync.dma_start(
                    out=yT_dram[b].rearrange("(c p) i -> p c i", p=P),
                    in_=yT[:])

                # te = exp(sigmoid(pid)); teT -> DRAM scratch (f32)
                pt = big_tile(F32, S)
                nc.sync.dma_start(
                    out=pt[:],
                    in_=pid_d[b].rearrange("(it p) j -> p it j", p=P))
                for it in range(NT):
                    nc.scalar.activation(out=pt[:, it, :], in_=pt[:, it, :],
                                         func=AF.Sigmoid)
                    nc.scalar.activation(out=pt[:, it, :], in_=pt[:, it, :],
                                         func=AF.Exp)
                teT = transpose_512(lambda it: pt[:, it, :], F32)
                nc.sync.dma_start(
                    out=teT_dram[b].rearrange("(c p) i -> p c i", p=P),
                    in_=teT[:])

                # forget gate, pre-scaled by 1/sqrt(DK)
                ft = frsp.tile([P, NT], F32, tag="frs")
                nc.sync.dma_start(
                    out=ft[:], in_=fr_d[b].rearrange("(t p) -> p t", p=P))
                nc.scalar.mul(out=ft[:], in_=ft[:], mul=1.0 / np.sqrt(DK))
                frs[b] = ft

            # ============ layers ============
            for l in range(L):
                wk = w3p.tile([P, DT, D], F32R, tag="w3")
                nc.sync.dma_start(
                    out=wk[:], in_=wk_d[l].rearrange("(c p) m -> p c m", p=P))
                wv = w3p.tile([P, DT, D], F32R, tag="w3")
                nc.sync.dma_start(
                    out=wv[:], in_=wv_d[l].rearrange("(c p) m -> p c m", p=P))
                wo = w3p.tile([P, DT, D], F32R, tag="w3")
                nc.sync.dma_start(
                    out=wo[:], in_=wo_d[l].rearrange("(c p) m -> p c m", p=P))

                if not fast:
                    bk_sb = smallp.tile([P, DT], F32, tag="bk")
                    nc.sync.dma_start(
                        out=bk_sb[:],
                        in_=bk_d[l].rearrange("(c p) -> p c", p=P))
                    row = smallp.tile([1, D], F32, tag="brow", bufs=2)
                    nc.sync.dma_start(out=row[:], in_=bv_d[l][None, :])
                    bv_bc = bcast_row(row[:])
                    row2 = smallp.tile([1, D], F32, tag="brow", bufs=2)
                    nc.sync.dma_start(out=row2[:], in_=bo_d[l][None, :])
                    bo_bc = bcast_row(row2[:])
                    row3 = smallp.tile([1, D], F32, tag="brow", bufs=2)
                    nc.sync.dma_start(out=row3[:], in_=b2_d[l][None, :])
                    b2_bc = bcast_row(row3[:])
                    b1_sb = smallp.tile([P, NKF], F32, tag="b1")
                    nc.sync.dma_start(
                        out=b1_sb[:],
                        in_=b1_d[l].rearrange("(c p) -> p c", p=P))
                    rg1 = smallp.tile([1, D], F32, tag="brow", bufs=2)
                    nc.sync.dma_start(out=rg1[:], in_=g1_d[l][None, :])
                    g1_bc = bcast_row(rg1[:])
                    rb1 = smallp.tile([1, D], F32, tag="brow", bufs=2)
                    nc.sync.dma_start(out=rb1[:], in_=gb1_d[l][None, :])
                    gb1_bc = bcast_row(rb1[:])
                    rg2 = smallp.tile([1, D], F32, tag="brow", bufs=2)
                    nc.sync.dma_start(out=rg2[:], in_=g2_d[l][None, :])
                    g2_bc = bcast_row(rg2[:])
                    rb2 = smallp.tile([1, D], F32, tag="brow", bufs=2)
                    nc.sync.dma_start(out=rb2[:], in_=gb2_d[l][None, :])
                    gb2_bc = bcast_row(rb2[:])
                else:
                    bk_sb = None
                    bv_bc = bo_bc = b2_bc = None
                    b1_sb = None
                    g1_bc = gb1_bc = g2_bc = gb2_bc = None

                # -------- attention phase --------
                def emit_scores(b, qkT, teT, h):
                    hp0 = (h % 2) * DK
                    qh = qkT[hp0:hp0 + DK, h // 2, :]
                    eTs = []
                    for tj in range(NT):
                        i0 = tj * P
                        ni = S - i0
                        sc_ps = psp.tile([P, S], F32, tag="psC", name="scps")
                        nc.tensor.matmul(
                            sc_ps[:, 0:ni], qh[:, i0:i0 + P], qh[:, i0:S],
                            start=True, stop=True)
                        sp = med_tile(F32)
                        nc.vector.scalar_tensor_tensor(
                            out=sp[:, 0:ni], in0=sc_ps[:, 0:ni],
                            scalar=frs[b][:, tj:tj + 1],
                            in1=teT[:, tj, i0:S],
                            op0=ALU.mult, op1=ALU.mult)
                        # strict causal mask on the diagonal block:
                        # keep j < i, i.e. partition p < free f
                        nc.gpsimd.affine_select(
                            out=sp[:, 0:P], in_=sp[:, 0:P],
                            compare_op=ALU.is_gt, fill=NEG_BIG,
                            base=0, channel_multiplier=-1,
                            pattern=[[1, P]])
                        eT = med_tile(F32R)
                        nc.scalar.activation(out=eT[:, 0:ni],
                                             in_=sp[:, 0:ni], func=AF.Exp)
                        eTs.append(eT)
                    return eTs

                def emit_pv(vext, ctxT, h, eTs):
                    hp0 = (h % 2) * DK
                    ctx_ps = psp.tile([P, S], F32, tag="psC", name="ctxps")
                    for tj in range(NT):
                        i0 = tj * P
                        ni = S - i0
                        nc.tensor.matmul(
                            ctx_ps[0:DK + 1, i0:S],
                            vext[:, tj, h, :], eTs[tj][:, 0:ni],
                            start=(tj == 0), stop=(tj == NT - 1))
                    dtmp = smallp.tile([1, S], F32, tag="dtmp", bufs=2)
                    nc.scalar.activation(
                        out=dtmp[:], in_=ctx_ps[DK:DK + 1, :],
                        func=AF.Identity, bias=eps37[0:1])
                    denB = smallp.tile([DK, S], F32, tag="dinvB", bufs=3)
                    nc.gpsimd.partition_broadcast(denB[:], dtmp[:])
                    dinvB = smallp.tile([DK, S], F32, tag="dinvB", bufs=3)
                    nc.vector.reciprocal_approx_fast(out=dinvB[:],
                                                     in_=denB[:])
                    nc.vector.tensor_mul(
                        out=ctxT[hp0:hp0 + DK, h // 2, :],
                        in0=ctx_ps[0:DK, :], in1=dinvB[:])

                for b in range(NB):
                    xb = x_tiles[b]
                    # prefetch the per-seq DRAM-scratch streams first so the
                    # DMAs overlap the transpose + projection matmuls
                    teT = big_tile(F32, S)
                    nc.gpsimd.dma_start(
                        out=teT[:],
                        in_=teT_dram[b].rearrange("(c p) i -> p c i", p=P))
                    yT = big_tile(F32R)
                    nc.gpsimd.dma_start(
                        out=yT[:],
                        in_=yT_dram[b].rearrange("(c p) i -> p c i", p=P))
                    xT = transpose_512(lambda it: xb[it], F32R)

                    # qkT[d, i] feature-major
                    qkT = big_tile(F32R)
                    for mt in range(DT):
                        ps = psp.tile([P, S], F32, tag="psC")
                        for c in range(DT):
                            nc.tensor.matmul(
                                ps[:], wk[:, c, mt * P:(mt + 1) * P],
                                xT[:, c, :], start=(c == 0),
                                stop=(c == DT - 1))
                        if bk_sb is not None:
                            nc.scalar.activation(
                                out=qkT[:, mt, :], in_=ps[:],
                                func=AF.Identity, bias=bk_sb[:, mt:mt + 1])
                        else:
                            nc.scalar.copy(out=qkT[:, mt, :], in_=ps[:])

                    # v seq-major with ones column per head: [128, it, h, 65]
                    vext = bigp.tile([P, NT, H, DK + 1], F32R, tag="big")
                    nc.scalar.copy(
                        out=vext[:, :, :, DK:DK + 1],
                        in_=ones32[:].rearrange("p (a b o) -> p a b o",
                                                a=NT, b=H, o=1))
                    for it in range(NT):
                        ps = psp.tile([P, S], F32, tag="psC")
                        for c in range(DT):
                            nc.tensor.matmul(
                                ps[:], yT[:, c, it * P:(it + 1) * P],
                                wv[:, c, :], start=(c == 0),
                                stop=(c == DT - 1))
                        pv = ps[:].rearrange("p (h k) -> p h k", h=H)
                        if bv_bc is not None:
                            nc.vector.scalar_tensor_tensor(
                                out=vext[:, it, :, 0:DK], in0=pv, scalar=1.0,
                                in1=bv_bc[:].rearrange("p (h k) -> p h k",
                                                       h=H),
                                op0=ALU.mult, op1=ALU.add)
                        else:
                            nc.vector.tensor_copy(out=vext[:, it, :, 0:DK],
                                                  in_=pv)

                    ctxT = big_tile(F32R)
                    from collections import deque
                    pending = deque()
                    for h in range(H):
                        pending.append((h, emit_scores(b, qkT, teT, h)))
                        if len(pending) > 3:
                            ph, peTs = pending.popleft()
                            emit_pv(vext, ctxT, ph, peTs)
                    while pending:
                        ph, peTs = pending.popleft()
                        emit_pv(vext, ctxT, ph, peTs)

                    # out-proj + residual + LN1
                    x1b = []
                    for it in range(NT):
                        ps = psp.tile([P, S], F32, tag="psC")
                        for c in range(DT):
                            nc.tensor.matmul(
                                ps[:], ctxT[:, c, it * P:(it + 1) * P],
                                wo[:, c, :], start=(c == 0),
                                stop=(c == DT - 1))
                        t = med_tile(F32)
                        rs = smallp.tile([P, 1], F32, tag="rs")
                        if bo_bc is not None:
                            nc.vector.scalar_tensor_tensor(
                                out=t[:], in0=ps[:], scalar=1.0, in1=bo_bc[:],
                                op0=ALU.mult, op1=ALU.add)
                            nc.vector.scalar_tensor_tensor(
                                out=t[:], in0=t[:], scalar=1.0, in1=xb[it][:],
                                op0=ALU.mult, op1=ALU.add, accum_out=rs)
                        else:
                            nc.vector.scalar_tensor_tensor(
                                out=t[:], in0=ps[:], scalar=1.0,
                                in1=xb[it][:],
                                op0=ALU.mult, op1=ALU.add, accum_out=rs)
                        x1 = statep.tile([P, D], F32, tag="x")
                        ln_apply(t[:], rs[:], x1[:], g1_bc and g1_bc[:],
                                 gb1_bc and gb1_bc[:])
                        x1b.append(x1)
                    x_tiles[b] = x1b

                # -------- FFN phase --------
                for b in range(NB):
                    x1b = x_tiles[b]
                    x1T = transpose_512(lambda it: x1b[it], FFN_DT)
                    y2_ps = [psp.tile([P, S], F32, tag="psC", name="y2ps")
                             for _i in range(NT)]
                    pend_ffn2 = []
                    for g in range(NKF // 4):
                        w1g = w1p.tile([P, DT, 4 * P], FFN_DT, tag="w1")
                        nc.sync.dma_start(
                            out=w1g[:],
                            in_=w1_d[l].rearrange("(c p) f -> p c f",
                                                  p=P)[:, :,
                                                       g * 512:(g + 1) * 512])
                        w2g = w2p.tile([P, 4, D], FFN_DT, tag="w2")
                        nc.sync.dma_start(
                            out=w2g[:],
                            in_=w2_d[l].rearrange("(c p) d -> p c d",
                                                  p=P)[:, 4 * g:4 * g + 4, :])
                        for j in range(4):
                            kf = 4 * g + j
                            h_ps = psp.tile([P, S], F32, tag="psC")
                            for c in range(DT):
                                nc.tensor.matmul(
                                    h_ps[:], w1g[:, c, j * P:(j + 1) * P],
                                    x1T[:, c, :], start=(c == 0),
                                    stop=(c == DT - 1))
                            hT = med_tile(FFN_DT)
                            if b1_sb is not None:
                                nc.scalar.activation(
                                    out=hT[:], in_=h_ps[:], func=AF.Relu,
                                    bias=b1_sb[:, kf:kf + 1])
                            elif kf % 2 == 0:
                                nc.scalar.activation(out=hT[:], in_=h_ps[:],
                                                     func=AF.Relu)
                            else:
                                nc.vector.tensor_scalar_max(
                                    out=hT[:], in0=h_ps[:], scalar1=0.0)
                            # pipeline by two kf: ffn2(kf-2) is emitted
                            # after ffn1(kf) so the PE isn't stalled on relu
                            pend_ffn2.append((hT, w2g, j, kf))
                            if len(pend_ffn2) > 2:
                                phT, pw2g, pj, pkf = pend_ffn2.pop(0)
                                for it in range(NT):
                                    nc.tensor.matmul(
                                        y2_ps[it][:],
                                        phT[:, it * P:(it + 1) * P],
                                        pw2g[:, pj, :], start=(pkf == 0),
                                        stop=(pkf == NKF - 1))
                    for phT, pw2g, pj, pkf in pend_ffn2:
                        for it in range(NT):
                            nc.tensor.matmul(
                                y2_ps[it][:], phT[:, it * P:(it + 1) * P],
                                pw2g[:, pj, :], start=(pkf == 0),
                                stop=(pkf == NKF - 1))
                    x2b = []
                    for it in range(NT):
                        t2 = med_tile(F32)
                        rs2 = smallp.tile([P, 1], F32, tag="rs")
                        if b2_bc is not None:
                            nc.vector.scalar_tensor_tensor(
                                out=t2[:], in0=y2_ps[it][:], scalar=1.0,
                                in1=b2_bc[:], op0=ALU.mult, op1=ALU.add)
                            nc.vector.scalar_tensor_tensor(
                                out=t2[:], in0=t2[:], scalar=1.0,
                                in1=x1b[it][:], op0=ALU.mult, op1=ALU.add,
                                accum_out=rs2)
                        else:
                            nc.vector.scalar_tensor_tensor(
                                out=t2[:], in0=y2_ps[it][:], scalar=1.0,
                                in1=x1b[it][:], op0=ALU.mult, op1=ALU.add,
                                accum_out=rs2)
                        x2 = statep.tile([P, D], F32, tag="x")
                        ln_apply(t2[:], rs2[:], x2[:], g2_bc and g2_bc[:],
                                 gb2_bc and gb2_bc[:])
                        x2b.append(x2)
                    x_tiles[b] = x2b
                    if l == L - 1:
                        for it in range(NT):
                            nc.sync.dma_start(
                                out=out_d[b, it * P:(it + 1) * P, :],
                                in_=x2b[it][:])

    nc.compile()
    return nc


_BUILD_CACHE = {}


def _get_nc(L, NB, fast):
    key = (L, NB, fast)
    if key not in _BUILD_CACHE:
        _BUILD_CACHE[key] = build(L, NB, fast)
    return _BUILD_CACHE[key]


def make_in_maps(inputs, L=4, NB=4, n_cores=N_CORES):
    """Shard full inputs into per-core in_maps. Returns (in_maps, fast)."""
    f32 = np.float32
    q = np.ascontiguousarray(np.asarray(inputs["q_embed_data"], f32))
    qa = np.ascontiguousarray(np.asarray(inputs["qa_embed_data"], f32))
    pid = np.ascontiguousarray(np.asarray(inputs["pid_embed_data"], f32))
    fr = np.ascontiguousarray(np.asarray(inputs["forget_rate"], f32)[:, :, 0])
    pos = np.ascontiguousarray(np.asarray(inputs["pos_emb"], f32)[0])
    names = ["Wk", "bk", "Wv", "bv", "Wo", "bo", "ln1_g", "ln1_b", "W1", "b1",
             "W2", "b2", "ln2_g", "ln2_b"]
    w = {n: np.ascontiguousarray(np.asarray(inputs[n], f32)) for n in names}
    if FFN_BF16:
        import ml_dtypes
        w["W1"] = w["W1"].astype(ml_dtypes.bfloat16)
        w["W2"] = w["W2"].astype(ml_dtypes.bfloat16)

    fast = (all(np.all(w[n] == 0.0) for n in
                ["bk", "bv", "bo", "b1", "b2", "ln1_b", "ln2_b"])
            and all(np.all(w[n] == 1.0) for n in ["ln1_g", "ln2_g"]))

    in_maps = []
    for c in range(n_cores):
        sl = slice(c * NB, (c + 1) * NB)
        m = {
            "q": q[sl], "qa": qa[sl], "pid": pid[sl], "fr": fr[sl],
            "pos": pos,
            "Wk": w["Wk"][:L], "Wv": w["Wv"][:L], "Wo": w["Wo"][:L],
            "W1": w["W1"][:L], "W2": w["W2"][:L],
        }
        if not fast:
            m.update({
                "bk": w["bk"][:L], "bv": w["bv"][:L], "bo": w["bo"][:L],
                "b1": w["b1"][:L], "b2": w["b2"][:L],
                "g1": w["ln1_g"][:L], "gb1": w["ln1_b"][:L],
                "g2": w["ln2_g"][:L], "gb2": w["ln2_b"][:L],
            })
        in_maps.append(m)
    return in_maps, fast


def kernel(**inputs):
    from concourse.bass_utils import run_bass_kernel_spmd

    B = int(np.asarray(inputs["q_embed_data"]).shape[0])
    NB = B // N_CORES
    L = int(np.asarray(inputs["Wk"]).shape[0])
    in_maps, fast = make_in_maps(inputs, L=L, NB=NB)
    nc = _get_nc(L, NB, fast)
    res = run_bass_kernel_spmd(nc, in_maps, core_ids=list(range(N_CORES)))
    out = np.concatenate([res.results[c]["out"] for c in range(N_CORES)],
                         axis=0)
    return out.astype(np.float32)



# revision 5
# speedup vs baseline: 1.2018x; 1.2018x over previous
"""Trainium2 Bass kernel for nn_DeepBKT (4-layer DeepBKT-style transformer).

Sharding: pure data-parallel over batch. B=32 sequences -> 8 NeuronCores x 4
sequences. Weights replicated. No collectives.

v2 design (vs v1 baseline at 1577us):
  - All matmul operands bf16 (stationaries get fast-weight-load, no f32r
    small-N penalty, LDWEIGHTS stream 4x lighter). State x kept in bf16;
    psum accumulation stays f32. Measured numpy rel err ~2e-3 (gate 2e-2).
  - Swapped PV: stationary = eT block [j,i-block], moving = v_ext [j,65]
    -> ctx lands [i, dk] with the softmax denominator as a per-partition
    COLUMN (ones-column trick). Kills the PartitionBroadcast + row-extract
    + wide-reciprocal + wide-multiply denominator pipeline of v1; the
    normalize folds into the psum-evacuation tensor_scalar.
  - forget-rate gate folded into the EXP activation's per-partition scale.
  - FFN weights DMA'd once per layer (v1 re-streamed per sequence: 128MB).
  - Attention(b) emission interleaved with projections of b+1 so the PE
    keeps running through the DVE/ACT-bound softmax stretches.
  - psum->sbuf evacuation copies spread across ACT/DVE/GpSimd by role.
"""

import sys

for _p in ("/opt/trn_rl_repo",):
    if _p not in sys.path:
        sys.path.insert(0, _p)

import numpy as np

import concourse.bacc as bacc
import concourse.bass as bass
import concourse.tile as tile
import concourse.mybir as mybir
from concourse.masks import make_identity

import concourse.tile_utils as tile_utils

tile_utils.max_sbuf_usage = 208 * 1024

F32 = mybir.dt.float32
F32R = mybir.dt.float32r
BF16 = mybir.dt.bfloat16
AF = mybir.ActivationFunctionType
ALU = mybir.AluOpType

P = 128
S, D, H, FF = 512, 512, 8, 2048
DK = D // H  # 64
NT = S // P  # 4 i/j tiles
DT = D // P  # 4 d tiles
NKF = FF // P  # 16 ff tiles
EPS = 1e-5
NEG_BIG = -1e30
N_CORES = 8


def build(L=4, NB=4):
    nc = bacc.Bacc("TRN2", target_bir_lowering=False, debug=False,
                   num_devices=N_CORES)

    q_d = nc.dram_tensor("q", [NB, S, D], F32, kind="ExternalInput")
    qa_d = nc.dram_tensor("qa", [NB, S, D], F32, kind="ExternalInput")
    pid_d = nc.dram_tensor("pid", [NB, S, S], F32, kind="ExternalInput")
    fr_d = nc.dram_tensor("fr", [NB, S], F32, kind="ExternalInput")
    pos_d = nc.dram_tensor("pos", [S, D], F32, kind="ExternalInput")
    wk_d = nc.dram_tensor("Wk", [L, D, D], BF16, kind="ExternalInput")
    wv_d = nc.dram_tensor("Wv", [L, D, D], BF16, kind="ExternalInput")
    wo_d = nc.dram_tensor("Wo", [L, D, D], BF16, kind="ExternalInput")
    w1_d = nc.dram_tensor("W1", [L, D, FF], BF16, kind="ExternalInput")
    w2_d = nc.dram_tensor("W2", [L, FF, D], BF16, kind="ExternalInput")
    out_d = nc.dram_tensor("out", [NB, S, D], F32, kind="ExternalOutput")

    with tile.TileContext(nc) as tc:
        with (
            tc.tile_pool(name="const", bufs=1) as constp,
            tc.tile_pool(name="state", bufs=1) as statep,
            tc.tile_pool(name="res", bufs=1) as resp,
            tc.tile_pool(name="wpool", bufs=1) as wp,
            tc.tile_pool(name="work", bufs=1) as workp,
            tc.tile_pool(name="bigf", bufs=2) as bigp,
            tc.tile_pool(name="small", bufs=6) as smallp,
            tc.tile_pool(name="ps", bufs=8, space="PSUM") as psp,
        ):
            identb = constp.tile([P, P], BF16, tag="identb")
            make_identity(nc, identb)
            eps_t = constp.tile([P, 1], F32, tag="eps")
            nc.vector.memset(eps_t, EPS)

            # ---------------- helpers ----------------
            def transpose4(src_of_it, dst, evac):
                """src_of_it(it) -> AP [128,512] bf16 (seq-major block).
                dst [128, DT, 512] bf16 feature-major. evac: 'act'|'dve'|'gp'"""
                for c in range(DT):
                    ps = psp.tile([P, S], BF16, tag="ps", name="tps")
                    for it in range(NT):
                        nc.tensor.transpose(
                            ps[:, it * P:(it + 1) * P],
                            src_of_it(it)[:, c * P:(c + 1) * P],
                            identb,
                        )
                    if evac == "act" or (evac == "mix" and c % 2 == 0):
                        nc.scalar.copy(out=dst[:, c, :], in_=ps[:])
                    else:
                        nc.vector.tensor_copy(out=dst[:, c, :], in_=ps[:])

            def ln_apply(t, rowsum, dst):
                """LayerNorm over free dim. t [128,512] bf16 pre-LN values,
                rowsum [128,1] f32 = sum over free. Writes normalized dst."""
                mean_neg = smallp.tile([P, 1], F32, tag="mneg")
                nc.scalar.mul(out=mean_neg, in_=rowsum, mul=-1.0 / D)
                var_s = smallp.tile([P, 1], F32, tag="vars")
                sq_scr = workp.tile([P, S], BF16, tag="sp", bufs=6,
                                    name="sqscr")
                nc.scalar.activation(out=sq_scr, in_=t, func=AF.Square,
                                     bias=mean_neg, scale=1.0,
                                     accum_out=var_s)
                std = smallp.tile([P, 1], F32, tag="std")
                nc.scalar.activation(out=std, in_=var_s, func=AF.Sqrt,
                                     bias=eps_t, scale=1.0 / D)
                rstd = smallp.tile([P, 1], F32, tag="rstd")
                nc.vector.reciprocal(out=rstd, in_=std)
                nc.vector.tensor_scalar(out=dst, in0=t, scalar1=mean_neg,
                                        scalar2=rstd, op0=ALU.add,
                                        op1=ALU.mult)

            # ---------------- resident state ----------------
            x_tiles = {}   # b -> [NT] state APs [128,512] bf16 seq-major
            yTs, teTs, frs = {}, {}, {}

            pos_t = bigp.tile([P, NT, D], F32, tag="big", name="post")
            nc.sync.dma_start(
                out=pos_t[:],
                in_=pos_d[:].rearrange("(it p) d -> p it d", p=P))

            for b in range(NB):
                qt = bigp.tile([P, NT, D], F32, tag="big", name="qt")
                nc.sync.dma_start(
                    out=qt[:], in_=q_d[b].rearrange("(it p) d -> p it d", p=P))
                xb = []
                for it in range(NT):
                    xt = statep.tile([P, D], BF16, tag="x", bufs=20, name="xt")
                    nc.vector.tensor_add(out=xt[:], in0=qt[:, it, :],
                                         in1=pos_t[:, it, :])
                    xb.append(xt)
                x_tiles[b] = xb

                yt = bigp.tile([P, NT, D], F32, tag="big", name="yt")
                nc.sync.dma_start(
                    out=yt[:], in_=qa_d[b].rearrange("(it p) d -> p it d", p=P))
                yb = workp.tile([P, NT, D], BF16, tag="eT", bufs=3, name="yb")
                for it in range(NT):
                    nc.vector.tensor_add(out=yb[:, it, :], in0=yt[:, it, :],
                                         in1=pos_t[:, it, :])
                yT = resp.tile([P, DT, S], BF16, tag="yT", bufs=NB, name="yT")
                transpose4(lambda it: yb[:, it, :], yT, "act")
                yTs[b] = yT

                pt = bigp.tile([P, NT, S], F32, tag="big", name="pt")
                nc.sync.dma_start(
                    out=pt[:],
                    in_=pid_d[b].rearrange("(it p) j -> p it j", p=P))
                ptb = workp.tile([P, NT, S], BF16, tag="eT", bufs=3,
                                 name="ptb")
                for it in range(NT):
                    nc.scalar.activation(out=pt[:, it, :], in_=pt[:, it, :],
                                         func=AF.Sigmoid)
                    nc.scalar.activation(out=ptb[:, it, :], in_=pt[:, it, :],
                                         func=AF.Exp)
                teT = resp.tile([P, NT, S], BF16, tag="teT", bufs=NB,
                                name="teT")
                transpose4(lambda it: ptb[:, it, :], teT, "dve")
                teTs[b] = teT

                ft = resp.tile([P, NT], F32, tag="frs", bufs=NB, name="ft")
                nc.sync.dma_start(
                    out=ft[:], in_=fr_d[b].rearrange("(t p) -> p t", p=P))
                nc.scalar.mul(out=ft[:], in_=ft[:], mul=1.0 / np.sqrt(DK))
                frs[b] = ft

            # ---------------- per-layer weights ----------------
            wk_t, wv_t, wo_t, w1_t, w2_t = {}, {}, {}, {}, {}

            def load_layer_weights(l):
                wk = wp.tile([P, DT, D], BF16, tag="w3", bufs=6, name="wk")
                nc.sync.dma_start(
                    out=wk[:], in_=wk_d[l].rearrange("(c p) m -> p c m", p=P))
                wv = wp.tile([P, DT, D], BF16, tag="w3", bufs=6, name="wv")
                nc.sync.dma_start(
                    out=wv[:], in_=wv_d[l].rearrange("(c p) m -> p c m", p=P))
                wo = wp.tile([P, DT, D], BF16, tag="w3", bufs=6, name="wo")
                nc.sync.dma_start(
                    out=wo[:], in_=wo_d[l].rearrange("(c p) m -> p c m", p=P))
                w1 = wp.tile([P, DT, FF], BF16, tag="w1", bufs=1, name="w1")
                nc.sync.dma_start(
                    out=w1[:], in_=w1_d[l].rearrange("(c p) f -> p c f", p=P))
                w2 = wp.tile([P, NKF, D], BF16, tag="w2", bufs=1, name="w2")
                nc.sync.dma_start(
                    out=w2[:], in_=w2_d[l].rearrange("(c p) d -> p c d", p=P))
                wk_t[l], wv_t[l], wo_t[l] = wk, wv, wo
                w1_t[l], w2_t[l] = w1, w2

            # ---------------- projection chunks (qkT, vext for (l,b)) ------
            proj_out = {}  # (l,b) -> (qkT, vext)

            def make_proj_chunks(l, b):
                """Returns list of closures; running all of them computes
                qkT[d,i] and vext[j,(h,dk+1)] for (l, b)."""
                xb = x_tiles[b]
                xT = workp.tile([P, DT, S], BF16, tag="xT", bufs=2, name="xT")
                qkT = workp.tile([P, DT, S], BF16, tag="qkT", bufs=2,
                                 name="qkT")
                vext = workp.tile([P, NT, H, DK + 1], BF16, tag="vext",
                                  bufs=2, name="vext")
                proj_out[(l, b)] = (qkT, vext)
                chunks = []

                def xt_chunk(c):
                    def run():
                        ps = psp.tile([P, S], BF16, tag="ps", name="xtps")
                        for it in range(NT):
                            nc.tensor.transpose(
                                ps[:, it * P:(it + 1) * P],
                                xb[it][:, c * P:(c + 1) * P], identb)
                        nc.vector.tensor_copy(out=xT[:, c, :], in_=ps[:])
                    return run

                def qk_chunk(mt):
                    def run():
                        ps = psp.tile([P, S], F32, tag="ps", name="qkps")
                        for c in range(DT):
                            nc.tensor.matmul(
                                ps[:], wk_t[l][:, c, mt * P:(mt + 1) * P],
                                xT[:, c, :], start=(c == 0),
                                stop=(c == DT - 1))
                        nc.scalar.copy(out=qkT[:, mt, :], in_=ps[:])
                    return run

                def v_chunk(it):
                    def run():
                        if it == 0:
                            nc.vector.memset(vext[:, :, :, DK:DK + 1], 1.0)
                        ps = psp.tile([P, S], F32, tag="ps", name="vps")
                        for c in range(DT):
                            nc.tensor.matmul(
                                ps[:], yTs[b][:, c, it * P:(it + 1) * P],
                                wv_t[l][:, c, :], start=(c == 0),
                                stop=(c == DT - 1))
                        nc.vector.tensor_copy(
                            out=vext[:, it, :, 0:DK],
                            in_=ps[:].rearrange("p (h k) -> p h k", h=H))
                    return run

                for c in range(DT):
                    chunks.append(xt_chunk(c))
                for mt in range(DT):
                    chunks.append(qk_chunk(mt))
                for it in range(NT):
                    chunks.append(v_chunk(it))
                return chunks

            # ---------------- attention for (l, b) ----------------
            def emit_scores(l, b, h):
                """-> eT tile [128, NT, 512] bf16 (j-major tiles)."""
                qkT, _ = proj_out[(l, b)]
                hp0 = (h % 2) * DK
                qh = qkT[hp0:hp0 + DK, h // 2, :]
                eT = workp.tile([P, NT, S], BF16, tag="eT", bufs=3, name="eT")
                for tj in range(NT):
                    i0 = tj * P
                    ni = S - i0
                    sc_ps = psp.tile([P, S], F32, tag="ps", name="scps")
                    nc.tensor.matmul(
                        sc_ps[:, 0:ni], qh[:, i0:i0 + P], qh[:, i0:S],
                        start=True, stop=True)
                    sp = workp.tile([P, S], BF16, tag="sp", bufs=6, name="sp")
                    nc.vector.tensor_mul(
                        out=sp[:, 0:ni], in0=sc_ps[:, 0:ni],
                        in1=teTs[b][:, tj, i0:S])
                    # strict causal mask on the diagonal block: keep j < i
                    nc.gpsimd.affine_select(
                        out=sp[:, 0:P], in_=sp[:, 0:P],
                        compare_op=ALU.is_gt, fill=NEG_BIG,
                        base=0, channel_multiplier=-1,
                        pattern=[[1, P]])
                    nc.scalar.activation(
                        out=eT[:, tj, 0:ni], in_=sp[:, 0:ni], func=AF.Exp,
                        scale=frs[b][:, tj:tj + 1])
                return eT

            def emit_pv(l, b, h, eT, ctxIH):
                """Swapped PV: ctx[i, dk] per i-tile with denominator column.
                Writes normalized ctx into ctxIH[ti][:, h*64:(h+1)*64]."""
                _, vext = proj_out[(l, b)]
                for ti in range(NT):
                    ctx_ps = psp.tile([P, DK + 1], F32, tag="ps", name="ctxps")
                    for tj in range(ti + 1):
                        nc.tensor.matmul(
                            ctx_ps[:],
                            eT[:, tj, (ti - tj) * P:(ti - tj) * P + P],
                            vext[:, tj, h, :],
                            start=(tj == 0), stop=(tj == ti))
                    den = smallp.tile([P, 1], F32, tag="den", name="den")
                    nc.vector.tensor_scalar_add(out=den,
                                                in0=ctx_ps[:, DK:DK + 1],
                                                scalar1=1e-37)
                    dinv = smallp.tile([P, 1], F32, tag="dinv", name="dinv")
                    nc.vector.reciprocal_approx_fast(out=dinv, in_=den)
                    nc.vector.tensor_scalar_mul(
                        out=ctxIH[ti][:, h * DK:(h + 1) * DK],
                        in0=ctx_ps[:, 0:DK], scalar1=dinv)

            def emit_attention(l, b, interleave):
                """Full attention for (l,b); pops closures from `interleave`
                between heads to keep the PE fed."""
                ctxIH = [workp.tile([P, D], BF16, tag="ctxIH", bufs=5,
                                    name="ctxIH") for _ in range(NT)]
                from collections import deque
                pending = deque()
                for h in range(H):
                    pending.append((h, emit_scores(l, b, h)))
                    if interleave:
                        interleave.pop(0)()
                    if len(pending) > 2:
                        ph, peT = pending.popleft()
                        emit_pv(l, b, ph, peT, ctxIH)
                while pending:
                    ph, peT = pending.popleft()
                    emit_pv(l, b, ph, peT, ctxIH)

                # ctx transposes -> ctxT feature-major
                ctxT = workp.tile([P, DT, S], BF16, tag="ctxT", bufs=1,
                                  name="ctxT")
                transpose4(lambda ti: ctxIH[ti][:], ctxT, "act")
                if interleave:
                    interleave.pop(0)()

                # out-proj + residual + LN1
                xb = x_tiles[b]
                x1b = []
                for it in range(NT):
                    ps = psp.tile([P, S], F32, tag="ps", name="oprps")
                    for c in range(DT):
                        nc.tensor.matmul(
                            ps[:], ctxT[:, c, it * P:(it + 1) * P],
                            wo_t[l][:, c, :], start=(c == 0),
                            stop=(c == DT - 1))
                    t = workp.tile([P, S], BF16, tag="t", bufs=6, name="t1")
                    rs = smallp.tile([P, 1], F32, tag="rs")
                    nc.vector.scalar_tensor_tensor(
                        out=t[:], in0=ps[:], scalar=1.0, in1=xb[it][:],
                        op0=ALU.mult, op1=ALU.add, accum_out=rs)
                    x1 = statep.tile([P, D], BF16, tag="x", bufs=20,
                                     name="x1")
                    ln_apply(t[:], rs[:], x1[:])
                    x1b.append(x1)
                    if interleave:
                        interleave.pop(0)()
                x_tiles[b] = x1b

                # x1T for the FFN
                x1T = workp.tile([P, DT, S], BF16, tag="x1T", bufs=1,
                                 name="x1T")
                transpose4(lambda it: x1b[it][:], x1T, "mix")
                while interleave:
                    interleave.pop(0)()
                return x1T

            # ---------------- FFN for (l, b) ----------------
            def emit_ffn(l, b, x1T, last):
                x1b = x_tiles[b]
                w1, w2 = w1_t[l], w2_t[l]
                y2_ps = [psp.tile([P, S], F32, tag="ps", name="y2ps")
                         for _ in range(NT)]
                pend = []
                for kf in range(NKF):
                    h_ps = psp.tile([P, S], F32, tag="ps", name="hps")
                    for c in range(DT):
                        nc.tensor.matmul(
                            h_ps[:], w1[:, c, kf * P:(kf + 1) * P],
                            x1T[:, c, :], start=(c == 0), stop=(c == DT - 1))
                    hT = workp.tile([P, S], BF16, tag="hT", bufs=3, name="hT")
                    if kf % 2 == 0:
                        nc.scalar.activation(out=hT[:], in_=h_ps[:],
                                             func=AF.Relu)
                    else:
                        nc.vector.tensor_scalar_max(out=hT[:], in0=h_ps[:],
                                                    scalar1=0.0)
                    pend.append((hT, kf))
                    if len(pend) > 2:
                        phT, pkf = pend.pop(0)
                        for it in range(NT):
                            nc.tensor.matmul(
                                y2_ps[it][:], phT[:, it * P:(it + 1) * P],
                                w2[:, pkf, :], start=(pkf == 0),
                                stop=(pkf == NKF - 1))
                for phT, pkf in pend:
                    for it in range(NT):
                        nc.tensor.matmul(
                            y2_ps[it][:], phT[:, it * P:(it + 1) * P],
                            w2[:, pkf, :], start=(pkf == 0),
                            stop=(pkf == NKF - 1))
                x2b = []
                for it in range(NT):
                    t2 = workp.tile([P, S], BF16, tag="t", bufs=6, name="t2")
                    rs2 = smallp.tile([P, 1], F32, tag="rs")
                    nc.vector.scalar_tensor_tensor(
                        out=t2[:], in0=y2_ps[it][:], scalar=1.0,
                        in1=x1b[it][:], op0=ALU.mult, op1=ALU.add,
                        accum_out=rs2)
                    if last:
                        x2 = statep.tile([P, D], F32, tag="xout", bufs=3,
                                         name="x2o")
                        ln_apply(t2[:], rs2[:], x2[:])
                        nc.sync.dma_start(
                            out=out_d[b, it * P:(it + 1) * P, :], in_=x2[:])
                    else:
                        x2 = statep.tile([P, D], BF16, tag="x", bufs=20,
                                         name="x2")
                        ln_apply(t2[:], rs2[:], x2[:])
                    x2b.append(x2)
                x_tiles[b] = x2b

            # ---------------- main schedule ----------------
            load_layer_weights(0)
            load_layer_weights(1)
            # prologue: projections for (0, 0) emitted standalone
            for ch in make_proj_chunks(0, 0):
                ch()
            for l in range(L):
                if 1 <= l and l + 1 < L:
                    load_layer_weights(l + 1)  # prefetch, overlaps compute
                for b in range(NB):
                    # projections of the NEXT (l, b) interleave into this
                    # attention + out-proj stretch
                    if b < NB - 1:
                        nxt = make_proj_chunks(l, b + 1)
                    elif l < L - 1:
                        nxt = make_proj_chunks(l + 1, 0)
                    else:
                        nxt = []
                    x1T = emit_attention(l, b, nxt)
                    emit_ffn(l, b, x1T, last=(l == L - 1))

    nc.compile()
    return nc


_BUILD_CACHE = {}


def _get_nc(L, NB):
    key = (L, NB)
    if key not in _BUILD_CACHE:
        _BUILD_CACHE[key] = build(L, NB)
    return _BUILD_CACHE[key]


def make_in_maps(inputs, L=4, NB=4, n_cores=N_CORES):
    """Shard full inputs into per-core in_maps."""
    import ml_dtypes
    f32 = np.float32
    bf = ml_dtypes.bfloat16
    q = np.ascontiguousarray(np.asarray(inputs["q_embed_data"], f32))
    qa = np.ascontiguousarray(np.asarray(inputs["qa_embed_data"], f32))
    pid = np.ascontiguousarray(np.asarray(inputs["pid_embed_data"], f32))
    fr = np.asarray(inputs["forget_rate"], f32)[:, :, 0]
    # guard: exact-zero forget rate would break the mask-fill folded into
    # the EXP scale (exp(0 * -1e30) = 1); reference gives uniform attention
    # over the past for fr == 0, which fr = 1e-20 reproduces.
    fr = np.ascontiguousarray(np.maximum(fr, 1e-20))
    pos = np.ascontiguousarray(np.asarray(inputs["pos_emb"], f32)[0])
    wdict = {}
    for n in ["Wk", "Wv", "Wo", "W1", "W2"]:
        wdict[n] = np.ascontiguousarray(
            np.asarray(inputs[n], f32).astype(bf))

    # biases / LN affine are zero/one in this model; verify and fall back
    # is not implemented (asserted host-side).
    for n in ["bk", "bv", "bo", "b1", "b2", "ln1_b", "ln2_b"]:
        assert np.all(np.asarray(inputs[n]) == 0.0), f"nonzero {n}"
    for n in ["ln1_g", "ln2_g"]:
        assert np.all(np.asarray(inputs[n]) == 1.0), f"non-unit {n}"

    in_maps = []
    for c in range(n_cores):
        sl = slice(c * NB, (c + 1) * NB)
        m = {
            "q": q[sl], "qa": qa[sl], "pid": pid[sl], "fr": fr[sl],
            "pos": pos,
            "Wk": wdict["Wk"][:L], "Wv": wdict["Wv"][:L],
            "Wo": wdict["Wo"][:L],
            "W1": wdict["W1"][:L], "W2": wdict["W2"][:L],
        }
        in_maps.append(m)
    return in_maps


def kernel(**inputs):
    from concourse.bass_utils import run_bass_kernel_spmd

    B = int(np.asarray(inputs["q_embed_data"]).shape[0])
    NB = B // N_CORES
    L = int(np.asarray(inputs["Wk"]).shape[0])
    in_maps = make_in_maps(inputs, L=L, NB=NB)
    nc = _get_nc(L, NB)
    res = run_bass_kernel_spmd(nc, in_maps, core_ids=list(range(N_CORES)))
    out = np.concatenate([res.results[c]["out"] for c in range(N_CORES)],
                         axis=0)
    return out.astype(np.float32)
